# revision 1
# baseline (speedup 1.0000x reference)
"""Trainium2 Bass kernel for nn_CnUpdateLayer (LDPC check-node update).

Math: out[b,i] = prod_{j: mask[i,j]!=0} x[b,j], or 0 if mask row i is empty.
Since mask is exactly {0,1} and x ~ randn (no exact zeros), the masked product
is computed in log-domain via matmul:

    L[b,i]  = sum_j ln|x[b,j]| * mask[i,j]       (magnitude, log domain)
    C[b,i]  = sum_j [x[b,j]<0] * mask[i,j]       (negative count)
    deg[i]  = sum_j mask[i,j]                    (row degree)
    out     = exp(L - 1000*[deg==0]) * (-1)^C

All three contractions share one pass of the mask through the PE array by
stacking [ln_hi | ln_lo | ones | signbits] as the stationary operand (97
columns).  ln|x| is split hi/lo into two bf16 halves so the matmul runs at
bf16 rate (1 cycle/row) while keeping ~fp32 accuracy; hi+lo sums are
recombined as exp(L_hi)*exp(L_lo).

Sharding: tensor-parallel over output edges (mask rows): core k owns output
columns [k*256, (k+1)*256); x is replicated.  Outputs are concatenated on
host - no collectives.  Host pre-swizzles both operands so every DMA is
contiguous per SBUF partition, and pre-casts the 0/1 mask to bf16 (exact).
"""

import sys

if "/opt/trn_rl_repo" not in sys.path:
    sys.path.insert(0, "/opt/trn_rl_repo")

import numpy as np

B = 32          # batch codewords
IN_F = 2048     # input edges
OUT_F = 2048    # output edges
NCORES = 8
SHARD = OUT_F // NCORES     # 256 output edges per core
KC = IN_F // 128            # 16 contraction chunks of 128
# lhsT column layout: PSUM reads must start 32-partition aligned, so the
# sign block sits at 64 and the ones/degree column at 96.
WHI, WLO, WSGN, WONE = 0, B, 2 * B, 3 * B       # 0, 32, 64, 96
WTOT = 3 * B + 1                                # 97

_PROG = None


def _build_program():
    import concourse.tile as tile
    from concourse import bacc, mybir
    from concourse.alu_op_type import AluOpType

    F32 = mybir.dt.float32
    BF16 = mybir.dt.bfloat16
    AF = mybir.ActivationFunctionType

    nc = bacc.Bacc("TRN2", target_bir_lowering=False)
    xt = nc.dram_tensor("xt", [128, KC * B], F32, kind="ExternalInput")
    mt = nc.dram_tensor("mt", [128, KC * SHARD], BF16, kind="ExternalInput")
    out = nc.dram_tensor("out", [B, SHARD], F32, kind="ExternalOutput")

    with tile.TileContext(nc) as tc:
        with (
            tc.tile_pool(name="pool", bufs=1) as pool,
            tc.tile_pool(name="psum", bufs=1, space="PSUM") as psum_pool,
        ):
            # ---- dummy Ln: hoists the 1.28us Ln ACT_TABLE_LOAD so it
            # overlaps the input DMAs.  (ACT tables reload on EVERY function
            # switch, so keep ACT strictly phased: Ln* then Exp*.)
            dmy = pool.tile([1, 1], F32)
            nc.vector.memset(dmy, 1.0)
            dln = pool.tile([1, 1], F32)
            nc.scalar.activation(out=dln, in_=dmy, func=AF.Ln)

            # constants (no deps, scheduled early)
            onesw = pool.tile([1, B], BF16)
            nc.vector.memset(onesw, 1.0)

            # tiny warm-up DMA on the (otherwise idle) scalar ring: spins up
            # all 16 SDMA engines (~1.3us activation ripple) before the real
            # x transfer lands.
            warm = pool.tile([128, 1], F32)
            nc.scalar.dma_start(out=warm, in_=xt.ap()[:, 0:1])

            # ---- input DMAs: one sync-ring queue, x FIRST (everything
            # downstream waits on it), then the mask in 4 groups of 4 chunks
            # that land just ahead of the PE's consumption.
            x_sb = pool.tile([128, KC, B], F32)
            nc.sync.dma_start(out=x_sb, in_=xt.ap().rearrange("p (c b) -> p c b", b=B))
            # mask in 4 groups of 4 chunks (2KB contiguous per partition per
            # group): measured to deliver the earliest first-group arrival,
            # which is what gates the matmul pipeline start.
            m_sb = pool.tile([128, KC, SHARD], BF16)
            mt_v = mt.ap().rearrange("p (c n) -> p c n", n=SHARD)
            DG = 4
            for g in range(0, KC, DG):
                nc.sync.dma_start(out=m_sb[:, g:g + DG, :], in_=mt_v[:, g:g + DG, :])

            # ---- stationary operand W = [hi | lo | sgn | ones], bf16.
            # ln|x| = ln(x^2) (x^2 on DVE avoids the Abs ACT table); the 0.5
            # is folded into the Exp scale.  Work is spread so no engine
            # exceeds the PE's ~0.85us/block cadence: DVE x^2+lo, ACT ln+hi
            # cast, GpSimd the sign bits.
            w_sb = pool.tile([128, KC, WTOT], BF16)
            nc.vector.memset(w_sb[:, :, WONE:WONE + 1], 1.0)
            sq_sb = pool.tile([128, KC, B], F32)
            ln_sb = pool.tile([128, KC, B], F32)
            PB = 4
            for h in range(0, KC, PB):
                sl = slice(h, h + PB)
                nc.vector.tensor_tensor(
                    out=sq_sb[:, sl, :], in0=x_sb[:, sl, :], in1=x_sb[:, sl, :],
                    op=AluOpType.mult)
                nc.scalar.activation(out=ln_sb[:, sl, :], in_=sq_sb[:, sl, :], func=AF.Ln)
                nc.scalar.copy(out=w_sb[:, sl, WHI:WHI + B], in_=ln_sb[:, sl, :])
                nc.vector.tensor_tensor(
                    out=w_sb[:, sl, WLO:WLO + B], in0=ln_sb[:, sl, :],
                    in1=w_sb[:, sl, WHI:WHI + B], op=AluOpType.subtract)
                nc.vector.tensor_scalar(
                    out=w_sb[:, sl, WSGN:WSGN + B], in0=x_sb[:, sl, :],
                    scalar1=0.0, scalar2=None, op0=AluOpType.is_lt)

            # dummy Exp AFTER the Ln phase (input reads ln_sb to pin the
            # ordering): its table load overlaps the matmuls instead of
            # stalling the real Exps.  ACT tables reload on every function
            # switch, so an early Exp would also force an Ln reload.
            dex = pool.tile([1, 1], F32)
            nc.scalar.activation(out=dex, in_=ln_sb[0:1, KC - 1, 0:1], func=AF.Exp)

            # ---- main accumulation: ps[0:97] += W_c^T @ M_c over 16 chunks ----
            ps = psum_pool.tile([128, SHARD], F32)
            for c in range(KC):
                nc.tensor.matmul(
                    ps[0:WTOT, :], lhsT=w_sb[:, c, :], rhs=m_sb[:, c, :],
                    start=(c == 0), stop=(c == KC - 1))

            # ---- epilogue: out = exp(.5*L_hi)*exp(.5*L_lo) * (min(degb,1)
            # - 2*p^2) where p = C - 2*rne(C/2) (parity via the 1.5*2^23
            # magic constant; mod is unsupported on DVE) and degb broadcasts
            # the degree row across batch partitions via a K=1 matmul.
            # Empty rows get p=0 and deg=0, hence exactly 0.
            # The PSUM bank tracker serializes ps-bank accessors pairwise in
            # trace order, so emit the short reads (Ccopy, z16) before the
            # Exps to keep the DVE parity chain unblocked.
            csb = pool.tile([B, SHARD], F32)
            nc.scalar.copy(out=csb, in_=ps[WSGN:WSGN + B, :])

            magh = pool.tile([B, SHARD], F32)
            nc.scalar.activation(out=magh, in_=ps[WHI:WHI + B, :], func=AF.Exp, scale=0.5)
            magl = pool.tile([B, SHARD], F32)
            nc.scalar.activation(out=magl, in_=ps[WLO:WLO + B, :], func=AF.Exp, scale=0.5)
            a = pool.tile([B, SHARD], F32)
            nc.gpsimd.tensor_tensor(out=a, in0=magh, in1=magl, op=AluOpType.mult)

            # zcopy after the Exps: it is not on the exp->a critical path,
            # only on the (shorter) K1-broadcast->v path.
            z16 = pool.tile([1, SHARD], BF16)
            nc.scalar.copy(out=z16, in_=ps[WONE:WONE + 1, :])  # deg<=16: exact bf16
            ps2 = psum_pool.tile([B, SHARD], F32)
            nc.tensor.matmul(ps2, lhsT=onesw, rhs=z16, start=True, stop=True)

            MAGIC = 12582912.0
            t = pool.tile([B, SHARD], F32)
            nc.vector.tensor_scalar(
                out=t, in0=csb, scalar1=0.5, scalar2=MAGIC,
                op0=AluOpType.mult, op1=AluOpType.add)
            r = pool.tile([B, SHARD], F32)
            nc.vector.tensor_scalar(
                out=r, in0=t, scalar1=MAGIC, scalar2=-2.0,
                op0=AluOpType.subtract, op1=AluOpType.mult)   # -2*rne(C/2)
            p = pool.tile([B, SHARD], F32)
            nc.vector.tensor_tensor(out=p, in0=r, in1=csb, op=AluOpType.add)
            u2 = pool.tile([B, SHARD], F32)
            nc.vector.scalar_tensor_tensor(
                out=u2, in0=p, scalar=-2.0, in1=p,
                op0=AluOpType.mult, op1=AluOpType.mult)       # -2*p^2
            v = pool.tile([B, SHARD], F32)
            nc.vector.scalar_tensor_tensor(
                out=v, in0=ps2, scalar=1.0, in1=u2,
                op0=AluOpType.min, op1=AluOpType.add)         # min(degb,1)-2p^2
            o_sb = pool.tile([B, SHARD], F32)
            nc.vector.tensor_tensor(out=o_sb, in0=a, in1=v, op=AluOpType.mult)
            nc.scalar.dma_start(out=out.ap(), in_=o_sb)

    nc.compile()
    return nc


def _get_program():
    global _PROG
    if _PROG is None:
        _PROG = _build_program()
    return _PROG


def _prep_inputs(x, mask):
    import ml_dtypes

    x = np.ascontiguousarray(x, dtype=np.float32)
    mask = np.ascontiguousarray(mask, dtype=np.float32)
    # xt[p, c*B + b] = x[b, c*128 + p]
    xt = np.ascontiguousarray(
        x.T.reshape(KC, 128, B).transpose(1, 0, 2).reshape(128, KC * B))
    mask_bf = mask.astype(ml_dtypes.bfloat16)
    in_maps = []
    for k in range(NCORES):
        shard = mask_bf[k * SHARD:(k + 1) * SHARD, :]      # [256, 2048]
        # mt[p, c*SHARD + n] = mask[k*SHARD + n, c*128 + p]
        mt = np.ascontiguousarray(
            shard.T.reshape(KC, 128, SHARD).transpose(1, 0, 2).reshape(128, KC * SHARD))
        in_maps.append({"xt": xt, "mt": mt})
    return in_maps


def run(x, mask, trace=False):
    """Run on 8 NeuronCores; returns (output, BassKernelResults)."""
    from concourse.bass_utils import run_bass_kernel_spmd

    nc = _get_program()
    in_maps = _prep_inputs(x, mask)
    res = run_bass_kernel_spmd(nc, in_maps, core_ids=list(range(NCORES)), trace=trace)
    out = np.concatenate([r["out"] for r in res.results], axis=1)
    return np.ascontiguousarray(out, dtype=np.float32), res


def kernel(x, mask):
    out, _ = run(x, mask, trace=False)
    return out

